# revision 14
# baseline (speedup 1.0000x reference)
"""Trainium2 Bass kernel for nn_AudioVideoInter (ragged_sequence).

Semantics (see reference): for each batch b,
  lab   = (labels[b] == 1)                       selection mask over T frames
  mean  = mean_c(video[:, b, :])                 per-frame channel mean  [T]
  vm    = compacted mean[lab]                    t selected means, in order
  scale[p] = prod_{m = max(0,p-T+t) .. min(p,t-1)} vm[m]
  out[:, b, :] = audio[:, b, :] * scale[:, None]

Closed form used on-device (with cq = forward cumprod over T of
w = (lab ? mean : 1), cr = backward cumprod of w, P = cq[T-1],
rank = exclusive cumsum of lab, t = sum(lab)):
  scale[p] = P                          for p in [t-1, T-t]
  scale[r] = cq[j_r]                    for selected j_r with rank r <= t-2
  scale[T-t+1+r] = P / cq[j_r] = cr[j_r + 1]     (same j_r)
Implemented as one gpsimd local_scatter of (value - P) into zeros, then +P.
Valid whenever 2t <= T+1 (t here is ~9..26, T=1024).

Sharding: pure data parallelism over batch. 8 cores x 4 batches each.
Within a core the 4 batches live at partitions {0,16,32,48} so the gpsimd
scatter is spread over 4 of the 8 Q7 cores.
"""

import os
import numpy as np

T, B, C = 1024, 32, 512
NCORES = 8
BL = B // NCORES          # batches per core = 4
NT = T // 128             # 8 tiles of 128 frames
SP = 16                   # partition stride between batches
PP = BL * SP              # 64 partitions used by the per-batch pipeline

_CACHE = {}
LAST_RESULT = None        # BassKernelResults of the most recent run (for test.py)


def _build_nc():
    import concourse.bass as bass
    import concourse.tile as tile
    from concourse import bacc, mybir
    from concourse.masks import make_identity

    f32 = mybir.dt.float32
    f16 = mybir.dt.float16
    i32 = mybir.dt.int32
    i16 = mybir.dt.int16
    Alu = mybir.AluOpType
    Ax = mybir.AxisListType

    nc = bacc.Bacc("TRN2", target_bir_lowering=False, debug=False)

    video = nc.dram_tensor("video_feat", [T, BL, C], f32, kind="ExternalInput").ap()
    audio = nc.dram_tensor("audio_feat", [T, BL, C], f32, kind="ExternalInput").ap()
    labels = nc.dram_tensor("labels", [BL, T], i32, kind="ExternalInput").ap()
    out = nc.dram_tensor("out", [T, BL, C], f32, kind="ExternalOutput").ap()

    ActFn = mybir.ActivationFunctionType

    with tile.TileContext(nc) as tc:
        with (
            tc.tile_pool(name="inb", bufs=10) as in_pool,
            tc.tile_pool(name="outp", bufs=4) as out_pool,
            tc.tile_pool(name="small", bufs=1) as small,
            tc.tile_pool(name="psum", bufs=2, space="PSUM") as psum,
        ):
            # ---- constants / init (gpsimd, off the DVE critical path) ----
            ident = small.tile([128, 128], f32)
            make_identity(nc, ident[:])
            lab_i = small.tile([PP, T], i32)
            nc.vector.memset(lab_i[:], 0)
            means_all = small.tile([128, NT, PP], f32)
            nc.vector.memset(means_all[:], 0.0)
            means_bT = small.tile([PP, T], f32)

            # ---- labels -> lab mask; batch b sits at partition SP*b ----
            lab_i_spread = lab_i[:].rearrange("(b s) t -> b s t", s=SP)[:, 0, :]
            nc.sync.dma_start(out=lab_i_spread, in_=labels)

            # ---- big-input DMAs. Video and audio share one pool/tag: slot
            # backpressure makes audio tile k's load wait for video tile
            # k-2's reduce, so video gets the DMA bandwidth first. ----
            vts = []
            for t in range(NT):
                vt = in_pool.tile([128, BL, C], f32, tag="inb")
                eng = nc.gpsimd if t < 3 else nc.sync
                eng.dma_start(out=vt[:], in_=video[t * 128 : (t + 1) * 128])
                vts.append(vt)
            ats = []
            for t in range(NT):
                at = in_pool.tile([128, BL, C], f32, tag="inb")
                nc.sync.dma_start(out=at[:], in_=audio[t * 128 : (t + 1) * 128])
                ats.append(at)

            # ---- label-only pipeline (ready before video finishes) ----
            lab_f = small.tile([PP, T], f32)
            nc.gpsimd.tensor_copy(out=lab_f[:], in_=lab_i[:])
            lab = small.tile([PP, T], f32)
            nc.gpsimd.tensor_single_scalar(
                out=lab[:], in_=lab_f[:], scalar=1.0, op=Alu.is_equal
            )
            t_cnt = small.tile([PP, 1], f32)
            nc.vector.tensor_reduce(out=t_cnt[:], in_=lab[:], axis=Ax.X, op=Alu.add)
            rank_i = small.tile([PP, T], f32)
            nc.vector.tensor_tensor_scan(
                out=rank_i[:], data0=lab[:], data1=lab[:], initial=0.0,
                op0=Alu.add, op1=Alu.bypass,
            )
            rank = small.tile([PP, T], f32)
            nc.gpsimd.tensor_sub(rank[:], rank_i[:], lab[:])
            tm1 = small.tile([PP, 1], f32)
            nc.gpsimd.tensor_single_scalar(
                out=tm1[:], in_=t_cnt[:], scalar=1.0, op=Alu.subtract
            )
            ofs = small.tile([PP, 1], f32)
            nc.gpsimd.tensor_scalar(
                out=ofs[:], in0=t_cnt[:], scalar1=-1.0, scalar2=float(T + 2),
                op0=Alu.mult, op1=Alu.add,
            )
            # maskA = (rank_i <= t-1) & lab   (== rank_excl <= t-2 for selected)
            c1 = small.tile([PP, T], f32)
            nc.gpsimd.tensor_single_scalar(
                out=c1[:], in_=rank_i[:], scalar=tm1[:], op=Alu.is_le
            )
            maskA = small.tile([PP, T], f32)
            nc.gpsimd.tensor_mul(maskA[:], c1[:], lab[:])
            idx_cat = small.tile([PP, 2 * T], i16)
            ra1 = small.tile([PP, T], f32)
            nc.gpsimd.tensor_single_scalar(
                out=ra1[:], in_=rank[:], scalar=1.0, op=Alu.add
            )
            qa = small.tile([PP, T], f32)
            nc.gpsimd.tensor_mul(qa[:], ra1[:], maskA[:])
            nc.gpsimd.tensor_single_scalar(
                out=idx_cat[:, 0:T], in_=qa[:], scalar=-1.0, op=Alu.add
            )
            rc = small.tile([PP, T], f32)
            nc.gpsimd.tensor_single_scalar(
                out=rc[:], in_=rank[:], scalar=ofs[:], op=Alu.add
            )
            qc = small.tile([PP, T], f32)
            nc.gpsimd.tensor_mul(qc[:], rc[:], maskA[:])
            nc.gpsimd.tensor_single_scalar(
                out=idx_cat[:, T : 2 * T], in_=qc[:], scalar=-1.0, op=Alu.add
            )

            # ---- per-frame channel sums + transpose to [b, T], and the
            # forward cumprod built incrementally per tile so only a short
            # tail remains after the last video tile lands. ----
            # Reduces split between DVE (tensor_reduce) and ACT (activation
            # accumulate) so phase 1 keeps pace with the video DMA stream.
            dummy = small.tile([128, C], f32)
            m1 = small.tile([PP, T], f32)
            w = small.tile([PP, T], f32)
            cq = small.tile([PP, T], f32)
            for t in range(NT):
                # channel sums for this 128-frame tile, written at stride SP
                means_sp = means_all[:].rearrange(
                    "p t (b s) -> p t b s", s=SP
                )
                if t in (2, 4, 7):
                    nc.vector.tensor_reduce(
                        out=means_sp[:, t, :, 0], in_=vts[t][:], axis=Ax.X,
                        op=Alu.add,
                    )
                else:
                    for b in range(BL):
                        nc.scalar.activation(
                            out=dummy[:], in_=vts[t][:, b, :], func=ActFn.Copy,
                            scale=1.0, accum_out=means_sp[:, t, b, 0:1],
                        )
                psum_mt = psum.tile([PP, 128], f32)
                nc.tensor.matmul(
                    psum_mt[:], means_all[:, t, :], ident[:], start=True, stop=True
                )
                sl = slice(t * 128, (t + 1) * 128)
                nc.vector.tensor_copy(out=means_bT[:, sl], in_=psum_mt[:])
                # w = (sum - C) * lab / C + 1   (folds the 1/C of the mean)
                nc.vector.scalar_tensor_tensor(
                    out=m1[:, sl], in0=means_bT[:, sl], scalar=-float(C),
                    in1=lab[:, sl], op0=Alu.add, op1=Alu.mult,
                )
                nc.vector.tensor_scalar(
                    out=w[:, sl], in0=m1[:, sl], scalar1=1.0 / C, scalar2=1.0,
                    op0=Alu.mult, op1=Alu.add,
                )
                init = 1.0 if t == 0 else cq[:, t * 128 - 1 : t * 128]
                nc.vector.tensor_tensor_scan(
                    out=cq[:, sl], data0=w[:, sl], data1=w[:, sl],
                    initial=init, op0=Alu.mult, op1=Alu.bypass,
                )

            P_ap = cq[:, T - 1 : T]
            # backward cumprod: cr[j] = prod_{j' >= j} w[j']   (reversed APs)
            cr = small.tile([PP, T], f32)
            nc.vector.tensor_tensor_scan(
                out=cr[:, ::-1], data0=w[:, ::-1], data1=w[:, ::-1], initial=1.0,
                op0=Alu.mult, op1=Alu.bypass,
            )

            # scatter data (value - P) in fp16, two scatters so scatter A
            # overlaps the backward scan / dataC computation on DVE
            data_a = small.tile([PP, T], f16)
            nc.vector.tensor_scalar(
                out=data_a[:], in0=cq[:], scalar1=P_ap, scalar2=None,
                op0=Alu.subtract,
            )
            dst_a = small.tile([PP, T], f16)
            nc.gpsimd.local_scatter(
                out_ap=dst_a[:], data_ap=data_a[:], idxs_ap=idx_cat[:, 0:T],
                channels=PP, num_elems=T, num_idxs=T,
            )
            # dataC[j] = cr[j+1] - P  (j = T-1 never scattered)
            data_c = small.tile([PP, T], f16)
            nc.vector.tensor_scalar(
                out=data_c[:, 0 : T - 1], in0=cr[:, 1:T], scalar1=P_ap,
                scalar2=None, op0=Alu.subtract,
            )
            nc.vector.memset(data_c[:, T - 1 : T], 0.0)
            dst_c = small.tile([PP, T], f16)
            nc.gpsimd.local_scatter(
                out_ap=dst_c[:], data_ap=data_c[:], idxs_ap=idx_cat[:, T : 2 * T],
                channels=PP, num_elems=T, num_idxs=T,
            )
            # scale = dst_a + dst_c + P
            scale_bT = small.tile([PP, T], f32)
            nc.vector.scalar_tensor_tensor(
                out=scale_bT[:], in0=dst_a[:], scalar=P_ap, in1=dst_c[:],
                op0=Alu.add, op1=Alu.add,
            )

            # ---- transpose scale back to [j, b] and multiply audio ----
            scale_jb = small.tile([128, NT, PP], f32)
            for t in range(NT):
                pst = psum.tile([128, PP], f32)
                nc.tensor.matmul(
                    pst[:],
                    scale_bT[:, t * 128 : (t + 1) * 128],
                    ident[0:PP, 0:PP],
                    start=True,
                    stop=True,
                )
                nc.vector.tensor_copy(out=scale_jb[:, t, :], in_=pst[:])

            for t in range(NT):
                ot = out_pool.tile([128, BL, C], f32)
                for b in range(BL):
                    s_ap = scale_jb[:, t, SP * b : SP * b + 1]
                    if b < BL // 2:
                        nc.vector.tensor_scalar_mul(
                            out=ot[:, b, :], in0=ats[t][:, b, :], scalar1=s_ap
                        )
                    else:
                        nc.scalar.mul(out=ot[:, b, :], in_=ats[t][:, b, :], mul=s_ap)
                nc.sync.dma_start(out=out[t * 128 : (t + 1) * 128], in_=ot[:])

    nc.compile()
    return nc


def _get_nc():
    if "nc" not in _CACHE:
        _CACHE["nc"] = _build_nc()
    return _CACHE["nc"]


def _ensure_ntff_hook():
    """The agent image's antenv lacks axon_hooks; provide it and register the
    ctypes-based NTFF profiling hook so trace=True works under axon."""
    import sys
    import types

    if "antenv.axon_hooks" in sys.modules:
        return
    mod = types.ModuleType("antenv.axon_hooks")
    state = {"hook": None}
    mod.set_axon_ntff_profile_hook = lambda h: state.__setitem__("hook", h)
    mod.get_axon_ntff_profile_hook = lambda: state["hook"]
    sys.modules["antenv.axon_hooks"] = mod
    try:
        from trn_agent_boot.trn_boot import _ntff_profile_via_ctypes

        so_path = "/opt/axon/libaxon_pjrt.so"
        if os.path.exists(so_path):
            mod.set_axon_ntff_profile_hook(_ntff_profile_via_ctypes(so_path))
    except Exception:
        pass


def kernel(video_feat: np.ndarray, audio_feat: np.ndarray, labels: np.ndarray) -> np.ndarray:
    global LAST_RESULT
    from concourse.bass_utils import run_bass_kernel_spmd

    video_feat = np.ascontiguousarray(video_feat, dtype=np.float32)
    audio_feat = np.ascontiguousarray(audio_feat, dtype=np.float32)
    labels = np.ascontiguousarray(labels, dtype=np.int32)

    nc = _get_nc()
    in_maps = []
    for m in range(NCORES):
        bs = slice(m * BL, (m + 1) * BL)
        in_maps.append(
            {
                "video_feat": np.ascontiguousarray(video_feat[:, bs, :]),
                "audio_feat": np.ascontiguousarray(audio_feat[:, bs, :]),
                "labels": np.ascontiguousarray(labels[bs, :]),
            }
        )

    trace = bool(os.environ.get("KERNEL_PROFILE"))
    if trace:
        _ensure_ntff_hook()
    kwargs = {}
    if trace and os.environ.get("KERNEL_PROFILE_ALL_CORES"):
        kwargs["trace_cores"] = list(range(NCORES))
    res = run_bass_kernel_spmd(
        nc, in_maps, core_ids=list(range(NCORES)), trace=trace, **kwargs
    )
    LAST_RESULT = res
    outs = [res.results[m]["out"] for m in range(NCORES)]
    return np.concatenate(outs, axis=1)


# revision 15
# speedup vs baseline: 1.8869x; 1.8869x over previous
"""Trainium2 Bass kernel for nn_AudioVideoInter (ragged_sequence).

Semantics (see reference): for each batch b,
  lab   = (labels[b] == 1)                       selection mask over T frames
  mean  = mean_c(video[:, b, :])                 per-frame channel mean  [T]
  vm    = compacted mean[lab]                    t selected means, in order
  scale[p] = prod_{m = max(0,p-T+t) .. min(p,t-1)} vm[m]
  out[:, b, :] = audio[:, b, :] * scale[:, None]

Closed form used on-device (with cq = forward cumprod over T of
w = (lab ? mean : 1), cr = backward cumprod of w, P = cq[T-1],
rank = exclusive cumsum of lab, t = sum(lab)):
  scale[p] = P                          for p in [t-1, T-t]
  scale[r] = cq[j_r]                    for selected j_r with rank r <= t-2
  scale[T-t+1+r] = P / cq[j_r] = cr[j_r + 1]     (same j_r)
Implemented as one gpsimd local_scatter of (value - P) into zeros, then +P.
Valid whenever 2t <= T+1 (t here is ~9..26, T=1024).

Sharding: pure data parallelism over batch. 8 cores x 4 batches each.
Within a core the 4 batches live at partitions {0,16,32,48} so the gpsimd
scatter is spread over 4 of the 8 Q7 cores.
"""

import os
import numpy as np

T, B, C = 1024, 32, 512
NCORES = 8
BL = B // NCORES          # batches per core = 4
NT = T // 128             # 8 tiles of 128 frames
SP = 16                   # partition stride between batches
PP = BL * SP              # 64 partitions used by the per-batch pipeline

_CACHE = {}
LAST_RESULT = None        # BassKernelResults of the most recent run (for test.py)


def _build_nc():
    import concourse.bass as bass
    import concourse.tile as tile
    from concourse import bacc, mybir
    from concourse.masks import make_identity

    f32 = mybir.dt.float32
    f16 = mybir.dt.float16
    i32 = mybir.dt.int32
    i16 = mybir.dt.int16
    Alu = mybir.AluOpType
    Ax = mybir.AxisListType

    nc = bacc.Bacc("TRN2", target_bir_lowering=False, debug=False)

    video = nc.dram_tensor("video_feat", [T, BL, C], f32, kind="ExternalInput").ap()
    audio = nc.dram_tensor("audio_feat", [T, BL, C], f32, kind="ExternalInput").ap()
    labels = nc.dram_tensor("labels", [BL, T], i32, kind="ExternalInput").ap()
    out = nc.dram_tensor("out", [T, BL, C], f32, kind="ExternalOutput").ap()

    ActFn = mybir.ActivationFunctionType

    with tile.TileContext(nc) as tc:
        with (
            tc.tile_pool(name="inb", bufs=10) as in_pool,
            tc.tile_pool(name="outp", bufs=4) as out_pool,
            tc.tile_pool(name="small", bufs=1) as small,
            tc.tile_pool(name="psum", bufs=2, space="PSUM") as psum,
        ):
            # ---- constants / init (gpsimd, off the DVE critical path) ----
            ident = small.tile([128, 128], f32)
            make_identity(nc, ident[:])
            lab_i = small.tile([PP, T], i32)
            nc.vector.memset(lab_i[:], 0)
            means_all = small.tile([128, NT, PP], f32)
            nc.vector.memset(means_all[:], 0.0)
            means_bT = small.tile([PP, T], f32)

            # ---- labels -> lab mask; batch b sits at partition SP*b ----
            lab_i_spread = lab_i[:].rearrange("(b s) t -> b s t", s=SP)[:, 0, :]
            nc.sync.dma_start(out=lab_i_spread, in_=labels)

            # ---- big-input DMAs. Video and audio share one pool/tag: slot
            # backpressure makes audio tile k's load wait for video tile
            # k-2's reduce, so video gets the DMA bandwidth first. ----
            vts = []
            for t in range(NT):
                vt = in_pool.tile([128, BL, C], f32, tag="inb")
                eng = nc.gpsimd if t < 3 else nc.sync
                eng.dma_start(out=vt[:], in_=video[t * 128 : (t + 1) * 128])
                vts.append(vt)
            ats = []
            for t in range(NT):
                at = in_pool.tile([128, BL, C], f32, tag="inb")
                nc.sync.dma_start(out=at[:], in_=audio[t * 128 : (t + 1) * 128])
                ats.append(at)

            # ---- label-only pipeline (ready before video finishes) ----
            lab_f = small.tile([PP, T], f32)
            nc.vector.tensor_copy(out=lab_f[:], in_=lab_i[:])
            lab = small.tile([PP, T], f32)
            nc.vector.tensor_single_scalar(
                out=lab[:], in_=lab_f[:], scalar=1.0, op=Alu.is_equal
            )
            t_cnt = small.tile([PP, 1], f32)
            nc.vector.tensor_reduce(out=t_cnt[:], in_=lab[:], axis=Ax.X, op=Alu.add)
            rank_i = small.tile([PP, T], f32)
            nc.vector.tensor_tensor_scan(
                out=rank_i[:], data0=lab[:], data1=lab[:], initial=0.0,
                op0=Alu.add, op1=Alu.bypass,
            )
            rank = small.tile([PP, T], f32)
            nc.vector.tensor_sub(rank[:], rank_i[:], lab[:])
            tm2 = small.tile([PP, 1], f32)
            nc.vector.tensor_single_scalar(
                out=tm2[:], in_=t_cnt[:], scalar=2.0, op=Alu.subtract
            )
            ofs = small.tile([PP, 1], f32)
            nc.vector.tensor_scalar(
                out=ofs[:], in0=t_cnt[:], scalar1=-1.0, scalar2=float(T + 2),
                op0=Alu.mult, op1=Alu.add,
            )
            maskA = small.tile([PP, T], f32)
            nc.vector.scalar_tensor_tensor(
                out=maskA[:], in0=rank[:], scalar=tm2[:], in1=lab[:],
                op0=Alu.is_le, op1=Alu.mult,
            )
            idx_cat = small.tile([PP, 2 * T], i16)
            qa = small.tile([PP, T], f32)
            nc.vector.scalar_tensor_tensor(
                out=qa[:], in0=rank[:], scalar=1.0, in1=maskA[:],
                op0=Alu.add, op1=Alu.mult,
            )
            nc.vector.tensor_single_scalar(
                out=idx_cat[:, 0:T], in_=qa[:], scalar=-1.0, op=Alu.add
            )
            qc = small.tile([PP, T], f32)
            nc.vector.scalar_tensor_tensor(
                out=qc[:], in0=rank[:], scalar=ofs[:], in1=maskA[:],
                op0=Alu.add, op1=Alu.mult,
            )
            nc.vector.tensor_single_scalar(
                out=idx_cat[:, T : 2 * T], in_=qc[:], scalar=-1.0, op=Alu.add
            )

            # ---- per-frame channel sums + transpose to [b, T], and the
            # forward cumprod built incrementally per tile so only a short
            # tail remains after the last video tile lands. ----
            # Reduces split between DVE (tensor_reduce) and ACT (activation
            # accumulate) so phase 1 keeps pace with the video DMA stream.
            dummy = small.tile([128, C], f32)
            m1 = small.tile([PP, T], f32)
            w = small.tile([PP, T], f32)
            cq = small.tile([PP, T], f32)
            for t in range(NT):
                # channel sums for this 128-frame tile, written at stride SP
                means_sp = means_all[:].rearrange(
                    "p t (b s) -> p t b s", s=SP
                )
                if t in (2, 4, 7):
                    nc.vector.tensor_reduce(
                        out=means_sp[:, t, :, 0], in_=vts[t][:], axis=Ax.X,
                        op=Alu.add,
                    )
                else:
                    for b in range(BL):
                        nc.scalar.activation(
                            out=dummy[:], in_=vts[t][:, b, :], func=ActFn.Copy,
                            scale=1.0, accum_out=means_sp[:, t, b, 0:1],
                        )
                psum_mt = psum.tile([PP, 128], f32)
                nc.tensor.matmul(
                    psum_mt[:], means_all[:, t, :], ident[:], start=True, stop=True
                )
                sl = slice(t * 128, (t + 1) * 128)
                nc.vector.tensor_copy(out=means_bT[:, sl], in_=psum_mt[:])
                # w = (sum - C) * lab / C + 1   (folds the 1/C of the mean)
                nc.vector.scalar_tensor_tensor(
                    out=m1[:, sl], in0=means_bT[:, sl], scalar=-float(C),
                    in1=lab[:, sl], op0=Alu.add, op1=Alu.mult,
                )
                nc.vector.tensor_scalar(
                    out=w[:, sl], in0=m1[:, sl], scalar1=1.0 / C, scalar2=1.0,
                    op0=Alu.mult, op1=Alu.add,
                )
                init = 1.0 if t == 0 else cq[:, t * 128 - 1 : t * 128]
                nc.vector.tensor_tensor_scan(
                    out=cq[:, sl], data0=w[:, sl], data1=w[:, sl],
                    initial=init, op0=Alu.mult, op1=Alu.bypass,
                )

            P_ap = cq[:, T - 1 : T]
            # backward cumprod: cr[j] = prod_{j' >= j} w[j']   (reversed APs)
            cr = small.tile([PP, T], f32)
            nc.vector.tensor_tensor_scan(
                out=cr[:, ::-1], data0=w[:, ::-1], data1=w[:, ::-1], initial=1.0,
                op0=Alu.mult, op1=Alu.bypass,
            )

            # scatter data (value - P) in fp16, two scatters so scatter A
            # overlaps the backward scan / dataC computation on DVE
            data_a = small.tile([PP, T], f16)
            nc.vector.tensor_scalar(
                out=data_a[:], in0=cq[:], scalar1=P_ap, scalar2=None,
                op0=Alu.subtract,
            )
            dst_a = small.tile([PP, T], f16)
            nc.gpsimd.local_scatter(
                out_ap=dst_a[:], data_ap=data_a[:], idxs_ap=idx_cat[:, 0:T],
                channels=PP, num_elems=T, num_idxs=T,
            )
            # dataC[j] = cr[j+1] - P  (j = T-1 never scattered)
            data_c = small.tile([PP, T], f16)
            nc.vector.tensor_scalar(
                out=data_c[:, 0 : T - 1], in0=cr[:, 1:T], scalar1=P_ap,
                scalar2=None, op0=Alu.subtract,
            )
            nc.vector.memset(data_c[:, T - 1 : T], 0.0)
            dst_c = small.tile([PP, T], f16)
            nc.gpsimd.local_scatter(
                out_ap=dst_c[:], data_ap=data_c[:], idxs_ap=idx_cat[:, T : 2 * T],
                channels=PP, num_elems=T, num_idxs=T,
            )
            # scale = dst_a + dst_c + P
            scale_bT = small.tile([PP, T], f32)
            nc.vector.scalar_tensor_tensor(
                out=scale_bT[:], in0=dst_a[:], scalar=P_ap, in1=dst_c[:],
                op0=Alu.add, op1=Alu.add,
            )

            # ---- transpose scale back to [j, b] and multiply audio ----
            scale_jb = small.tile([128, NT, PP], f32)
            for t in range(NT):
                pst = psum.tile([128, PP], f32)
                nc.tensor.matmul(
                    pst[:],
                    scale_bT[:, t * 128 : (t + 1) * 128],
                    ident[0:PP, 0:PP],
                    start=True,
                    stop=True,
                )
                nc.vector.tensor_copy(out=scale_jb[:, t, :], in_=pst[:])

            for t in range(NT):
                ot = out_pool.tile([128, BL, C], f32)
                for b in range(BL):
                    s_ap = scale_jb[:, t, SP * b : SP * b + 1]
                    if b < BL // 2:
                        nc.vector.tensor_scalar_mul(
                            out=ot[:, b, :], in0=ats[t][:, b, :], scalar1=s_ap
                        )
                    else:
                        nc.scalar.mul(out=ot[:, b, :], in_=ats[t][:, b, :], mul=s_ap)
                nc.sync.dma_start(out=out[t * 128 : (t + 1) * 128], in_=ot[:])

    nc.compile()
    return nc


def _get_nc():
    if "nc" not in _CACHE:
        _CACHE["nc"] = _build_nc()
    return _CACHE["nc"]


def _ensure_ntff_hook():
    """The agent image's antenv lacks axon_hooks; provide it and register the
    ctypes-based NTFF profiling hook so trace=True works under axon."""
    import sys
    import types

    if "antenv.axon_hooks" in sys.modules:
        return
    mod = types.ModuleType("antenv.axon_hooks")
    state = {"hook": None}
    mod.set_axon_ntff_profile_hook = lambda h: state.__setitem__("hook", h)
    mod.get_axon_ntff_profile_hook = lambda: state["hook"]
    sys.modules["antenv.axon_hooks"] = mod
    try:
        from trn_agent_boot.trn_boot import _ntff_profile_via_ctypes

        so_path = "/opt/axon/libaxon_pjrt.so"
        if os.path.exists(so_path):
            mod.set_axon_ntff_profile_hook(_ntff_profile_via_ctypes(so_path))
    except Exception:
        pass


def kernel(video_feat: np.ndarray, audio_feat: np.ndarray, labels: np.ndarray) -> np.ndarray:
    global LAST_RESULT
    from concourse.bass_utils import run_bass_kernel_spmd

    video_feat = np.ascontiguousarray(video_feat, dtype=np.float32)
    audio_feat = np.ascontiguousarray(audio_feat, dtype=np.float32)
    labels = np.ascontiguousarray(labels, dtype=np.int32)

    nc = _get_nc()
    in_maps = []
    for m in range(NCORES):
        bs = slice(m * BL, (m + 1) * BL)
        in_maps.append(
            {
                "video_feat": np.ascontiguousarray(video_feat[:, bs, :]),
                "audio_feat": np.ascontiguousarray(audio_feat[:, bs, :]),
                "labels": np.ascontiguousarray(labels[bs, :]),
            }
        )

    trace = bool(os.environ.get("KERNEL_PROFILE"))
    if trace:
        _ensure_ntff_hook()
    kwargs = {}
    if trace and os.environ.get("KERNEL_PROFILE_ALL_CORES"):
        kwargs["trace_cores"] = list(range(NCORES))
    res = run_bass_kernel_spmd(
        nc, in_maps, core_ids=list(range(NCORES)), trace=trace, **kwargs
    )
    LAST_RESULT = res
    outs = [res.results[m]["out"] for m in range(NCORES)]
    return np.concatenate(outs, axis=1)


# revision 16
# speedup vs baseline: 1.9421x; 1.0293x over previous
"""Trainium2 Bass kernel for nn_AudioVideoInter (ragged_sequence).

Semantics (see reference): for each batch b,
  lab   = (labels[b] == 1)                       selection mask over T frames
  mean  = mean_c(video[:, b, :])                 per-frame channel mean  [T]
  vm    = compacted mean[lab]                    t selected means, in order
  scale[p] = prod_{m = max(0,p-T+t) .. min(p,t-1)} vm[m]
  out[:, b, :] = audio[:, b, :] * scale[:, None]

Closed form used on-device (with cq = forward cumprod over T of
w = (lab ? mean : 1), cr = backward cumprod of w, P = cq[T-1],
rank = exclusive cumsum of lab, t = sum(lab)):
  scale[p] = P                          for p in [t-1, T-t]
  scale[r] = cq[j_r]                    for selected j_r with rank r <= t-2
  scale[T-t+1+r] = P / cq[j_r] = cr[j_r + 1]     (same j_r)
Implemented as one gpsimd local_scatter of (value - P) into zeros, then +P.
Valid whenever 2t <= T+1 (t here is ~9..26, T=1024).

Sharding: pure data parallelism over batch. 8 cores x 4 batches each.
Within a core the 4 batches live at partitions {0,16,32,48} so the gpsimd
scatter is spread over 4 of the 8 Q7 cores.
"""

import os
import numpy as np

T, B, C = 1024, 32, 512
NCORES = 8
BL = B // NCORES          # batches per core = 4
NT = T // 128             # 8 tiles of 128 frames
SP = 16                   # partition stride between batches
PP = BL * SP              # 64 partitions used by the per-batch pipeline

_CACHE = {}
LAST_RESULT = None        # BassKernelResults of the most recent run (for test.py)


def _build_nc():
    import concourse.bass as bass
    import concourse.tile as tile
    from concourse import bacc, mybir
    from concourse.masks import make_identity

    f32 = mybir.dt.float32
    f16 = mybir.dt.float16
    i32 = mybir.dt.int32
    i16 = mybir.dt.int16
    Alu = mybir.AluOpType
    Ax = mybir.AxisListType

    nc = bacc.Bacc("TRN2", target_bir_lowering=False, debug=False)

    video = nc.dram_tensor("video_feat", [T, BL, C], f32, kind="ExternalInput").ap()
    audio = nc.dram_tensor("audio_feat", [T, BL, C], f32, kind="ExternalInput").ap()
    labels = nc.dram_tensor("labels", [BL, T], i32, kind="ExternalInput").ap()
    out = nc.dram_tensor("out", [T, BL, C], f32, kind="ExternalOutput").ap()

    ActFn = mybir.ActivationFunctionType

    with tile.TileContext(nc) as tc:
        with (
            tc.tile_pool(name="inb", bufs=10) as in_pool,
            tc.tile_pool(name="outp", bufs=4) as out_pool,
            tc.tile_pool(name="small", bufs=1) as small,
            tc.tile_pool(name="psum", bufs=2, space="PSUM") as psum,
        ):
            # ---- constants / init (gpsimd, off the DVE critical path) ----
            ident = small.tile([128, 128], f32)
            make_identity(nc, ident[:])
            lab_i = small.tile([PP, T], i32)
            nc.vector.memset(lab_i[:], 0)
            means_all = small.tile([128, NT, PP], f32)
            nc.vector.memset(means_all[:], 0.0)
            means_bT = small.tile([PP, T], f32)

            # ---- labels -> lab mask; batch b sits at partition SP*b ----
            lab_i_spread = lab_i[:].rearrange("(b s) t -> b s t", s=SP)[:, 0, :]
            nc.sync.dma_start(out=lab_i_spread, in_=labels)

            # ---- big-input DMAs. Video and audio share one pool/tag: slot
            # backpressure makes audio tile k's load wait for video tile
            # k-2's reduce, so video gets the DMA bandwidth first. ----
            vts = []
            for t in range(NT):
                vt = in_pool.tile([128, BL, C], f32, tag="inb")
                eng = nc.gpsimd if t < 3 else nc.sync
                eng.dma_start(out=vt[:], in_=video[t * 128 : (t + 1) * 128])
                vts.append(vt)
            ats = []
            for t in range(NT):
                at = in_pool.tile([128, BL, C], f32, tag="inb")
                nc.sync.dma_start(out=at[:], in_=audio[t * 128 : (t + 1) * 128])
                ats.append(at)

            # ---- label-only pipeline (ready before video finishes) ----
            lab_f = small.tile([PP, T], f32)
            nc.vector.tensor_copy(out=lab_f[:], in_=lab_i[:])
            lab = small.tile([PP, T], f32)
            nc.vector.tensor_single_scalar(
                out=lab[:], in_=lab_f[:], scalar=1.0, op=Alu.is_equal
            )
            t_cnt = small.tile([PP, 1], f32)
            nc.vector.tensor_reduce(out=t_cnt[:], in_=lab[:], axis=Ax.X, op=Alu.add)
            rank_i = small.tile([PP, T], f32)
            nc.vector.tensor_tensor_scan(
                out=rank_i[:], data0=lab[:], data1=lab[:], initial=0.0,
                op0=Alu.add, op1=Alu.bypass,
            )
            rank = small.tile([PP, T], f32)
            nc.vector.tensor_sub(rank[:], rank_i[:], lab[:])
            tm2 = small.tile([PP, 1], f32)
            nc.vector.tensor_single_scalar(
                out=tm2[:], in_=t_cnt[:], scalar=2.0, op=Alu.subtract
            )
            ofs = small.tile([PP, 1], f32)
            nc.vector.tensor_scalar(
                out=ofs[:], in0=t_cnt[:], scalar1=-1.0, scalar2=float(T + 2),
                op0=Alu.mult, op1=Alu.add,
            )
            maskA = small.tile([PP, T], f32)
            nc.vector.scalar_tensor_tensor(
                out=maskA[:], in0=rank[:], scalar=tm2[:], in1=lab[:],
                op0=Alu.is_le, op1=Alu.mult,
            )
            idx_cat = small.tile([PP, 2 * T], i16)
            qa = small.tile([PP, T], f32)
            nc.vector.scalar_tensor_tensor(
                out=qa[:], in0=rank[:], scalar=1.0, in1=maskA[:],
                op0=Alu.add, op1=Alu.mult,
            )
            nc.vector.tensor_single_scalar(
                out=idx_cat[:, 0:T], in_=qa[:], scalar=-1.0, op=Alu.add
            )
            qc = small.tile([PP, T], f32)
            nc.vector.scalar_tensor_tensor(
                out=qc[:], in0=rank[:], scalar=ofs[:], in1=maskA[:],
                op0=Alu.add, op1=Alu.mult,
            )
            nc.vector.tensor_single_scalar(
                out=idx_cat[:, T : 2 * T], in_=qc[:], scalar=-1.0, op=Alu.add
            )

            # ---- per-frame channel sums + transpose to [b, T], and the
            # forward cumprod built incrementally per tile so only a short
            # tail remains after the last video tile lands. ----
            # Reduces split between DVE (tensor_reduce) and ACT (activation
            # accumulate) so phase 1 keeps pace with the video DMA stream.
            dummy = small.tile([128, C], f32)
            m1 = small.tile([PP, T], f32)
            w = small.tile([PP, T], f32)
            cq = small.tile([PP, T], f32)
            for t in range(NT):
                # channel sums for this 128-frame tile, written at stride SP
                means_sp = means_all[:].rearrange(
                    "p t (b s) -> p t b s", s=SP
                )
                if t in (0, 2, 4, 7):
                    nc.vector.tensor_reduce(
                        out=means_sp[:, t, :, 0], in_=vts[t][:], axis=Ax.X,
                        op=Alu.add,
                    )
                else:
                    for b in range(BL):
                        nc.scalar.activation(
                            out=dummy[:], in_=vts[t][:, b, :], func=ActFn.Copy,
                            scale=1.0, accum_out=means_sp[:, t, b, 0:1],
                        )
                psum_mt = psum.tile([PP, 128], f32)
                nc.tensor.matmul(
                    psum_mt[:], means_all[:, t, :], ident[:], start=True, stop=True
                )
                sl = slice(t * 128, (t + 1) * 128)
                nc.vector.tensor_copy(out=means_bT[:, sl], in_=psum_mt[:])
                # w = (sum - C) * lab / C + 1   (folds the 1/C of the mean)
                nc.vector.scalar_tensor_tensor(
                    out=m1[:, sl], in0=means_bT[:, sl], scalar=-float(C),
                    in1=lab[:, sl], op0=Alu.add, op1=Alu.mult,
                )
                nc.vector.tensor_scalar(
                    out=w[:, sl], in0=m1[:, sl], scalar1=1.0 / C, scalar2=1.0,
                    op0=Alu.mult, op1=Alu.add,
                )
                init = 1.0 if t == 0 else cq[:, t * 128 - 1 : t * 128]
                nc.vector.tensor_tensor_scan(
                    out=cq[:, sl], data0=w[:, sl], data1=w[:, sl],
                    initial=init, op0=Alu.mult, op1=Alu.bypass,
                )

            P_ap = cq[:, T - 1 : T]
            # backward cumprod: cr[j] = prod_{j' >= j} w[j']   (reversed APs)
            cr = small.tile([PP, T], f32)
            nc.vector.tensor_tensor_scan(
                out=cr[:, ::-1], data0=w[:, ::-1], data1=w[:, ::-1], initial=1.0,
                op0=Alu.mult, op1=Alu.bypass,
            )

            # scatter data (value - P) in fp16, two scatters so scatter A
            # overlaps the backward scan / dataC computation on DVE
            data_a = small.tile([PP, T], f16)
            nc.vector.tensor_scalar(
                out=data_a[:], in0=cq[:], scalar1=P_ap, scalar2=None,
                op0=Alu.subtract,
            )
            dst_a = small.tile([PP, T], f16)
            nc.gpsimd.local_scatter(
                out_ap=dst_a[:], data_ap=data_a[:], idxs_ap=idx_cat[:, 0:T],
                channels=PP, num_elems=T, num_idxs=T,
            )
            # dataC[j] = cr[j+1] - P  (j = T-1 never scattered)
            data_c = small.tile([PP, T], f16)
            nc.vector.tensor_scalar(
                out=data_c[:, 0 : T - 1], in0=cr[:, 1:T], scalar1=P_ap,
                scalar2=None, op0=Alu.subtract,
            )
            nc.vector.memset(data_c[:, T - 1 : T], 0.0)
            dst_c = small.tile([PP, T], f16)
            nc.gpsimd.local_scatter(
                out_ap=dst_c[:], data_ap=data_c[:], idxs_ap=idx_cat[:, T : 2 * T],
                channels=PP, num_elems=T, num_idxs=T,
            )
            # scale = dst_a + dst_c + P
            scale_bT = small.tile([PP, T], f32)
            nc.vector.scalar_tensor_tensor(
                out=scale_bT[:], in0=dst_a[:], scalar=P_ap, in1=dst_c[:],
                op0=Alu.add, op1=Alu.add,
            )

            # ---- transpose scale back to [j, b] and multiply audio ----
            scale_jb = small.tile([128, NT, PP], f32)
            for t in range(NT):
                pst = psum.tile([128, PP], f32)
                nc.tensor.matmul(
                    pst[:],
                    scale_bT[:, t * 128 : (t + 1) * 128],
                    ident[0:PP, 0:PP],
                    start=True,
                    stop=True,
                )
                nc.vector.tensor_copy(out=scale_jb[:, t, :], in_=pst[:])

            for t in range(NT):
                ot = out_pool.tile([128, BL, C], f32)
                for b in range(BL):
                    s_ap = scale_jb[:, t, SP * b : SP * b + 1]
                    if b < BL // 2:
                        nc.vector.tensor_scalar_mul(
                            out=ot[:, b, :], in0=ats[t][:, b, :], scalar1=s_ap
                        )
                    else:
                        nc.scalar.mul(out=ot[:, b, :], in_=ats[t][:, b, :], mul=s_ap)
                nc.sync.dma_start(out=out[t * 128 : (t + 1) * 128], in_=ot[:])

    nc.compile()
    return nc


def _get_nc():
    if "nc" not in _CACHE:
        _CACHE["nc"] = _build_nc()
    return _CACHE["nc"]


def _ensure_ntff_hook():
    """The agent image's antenv lacks axon_hooks; provide it and register the
    ctypes-based NTFF profiling hook so trace=True works under axon."""
    import sys
    import types

    if "antenv.axon_hooks" in sys.modules:
        return
    mod = types.ModuleType("antenv.axon_hooks")
    state = {"hook": None}
    mod.set_axon_ntff_profile_hook = lambda h: state.__setitem__("hook", h)
    mod.get_axon_ntff_profile_hook = lambda: state["hook"]
    sys.modules["antenv.axon_hooks"] = mod
    try:
        from trn_agent_boot.trn_boot import _ntff_profile_via_ctypes

        so_path = "/opt/axon/libaxon_pjrt.so"
        if os.path.exists(so_path):
            mod.set_axon_ntff_profile_hook(_ntff_profile_via_ctypes(so_path))
    except Exception:
        pass


def kernel(video_feat: np.ndarray, audio_feat: np.ndarray, labels: np.ndarray) -> np.ndarray:
    global LAST_RESULT
    from concourse.bass_utils import run_bass_kernel_spmd

    video_feat = np.ascontiguousarray(video_feat, dtype=np.float32)
    audio_feat = np.ascontiguousarray(audio_feat, dtype=np.float32)
    labels = np.ascontiguousarray(labels, dtype=np.int32)

    nc = _get_nc()
    in_maps = []
    for m in range(NCORES):
        bs = slice(m * BL, (m + 1) * BL)
        in_maps.append(
            {
                "video_feat": np.ascontiguousarray(video_feat[:, bs, :]),
                "audio_feat": np.ascontiguousarray(audio_feat[:, bs, :]),
                "labels": np.ascontiguousarray(labels[bs, :]),
            }
        )

    trace = bool(os.environ.get("KERNEL_PROFILE"))
    if trace:
        _ensure_ntff_hook()
    kwargs = {}
    if trace and os.environ.get("KERNEL_PROFILE_ALL_CORES"):
        kwargs["trace_cores"] = list(range(NCORES))
    res = run_bass_kernel_spmd(
        nc, in_maps, core_ids=list(range(NCORES)), trace=trace, **kwargs
    )
    LAST_RESULT = res
    outs = [res.results[m]["out"] for m in range(NCORES)]
    return np.concatenate(outs, axis=1)
